# revision 1
# baseline (speedup 1.0000x reference)
"""Trainium2 Bass kernel for nn_MirrorDescentLinear.

Reference computation:
    w[o,i] = (e1 - e0) / (1 + e0 + e1)            (softmax(+1) - softmax(-1))
    w *= bf16(scales)[o, i//128]                   (per-group scale)
    w *= mask[o,i]                                 (0/1 int mask)
    y = x @ w.T                                    (f32, [8192,4096]@[4096,4096].T)

Sharding (8 cores): tensor-parallel 4-way on out_features x 2-way on tokens.
Each core computes y[t_half, o_quarter] from logits/scales/mask[o_quarter]
and xT[:, t_half]. The host pre-transposes x once (layout-only) so the
contraction dim I lands on SBUF partitions without any on-chip transpose of x.

Per-core device program:
  phase A (weights): exp on ScalarE; 1/d as exp(-ln d) on ScalarE; e1-e0,
    int-mask multiply, stride-0-broadcast group-scale multiply and recip
    multiply on VectorE; w tiles transposed on TensorE (4 per PSUM bank,
    single-copy evacuation) into resident wT[i, o] float32r tiles, one per
    512-wide i-chunk so phase B can start consuming early chunks.
  phase B (matmul): float32r matmuls (full-rate at N=512, FP22 mantissa)
    accumulating over 32 i-chunks into PSUM; VectorE evacuates, DMA stores y.

Measured on trn2 (single-core loop bench, host I/O excluded): ~740 us/core.
"""

import json
import sys

sys.path.insert(0, "/opt/trn_rl_repo")

import numpy as np

import concourse.bass as bass
import concourse.tile as tile
from concourse import mybir
from concourse.bass_utils import run_bass_kernel_spmd
from concourse.masks import make_identity
from concourse.tile_scheduler import N_PROCS
from concourse.vector_clock import ScopedClock, VectorClock

# ---------------------------------------------------------------------------
# Compatibility patches for the bundled walrus (accepts at most ONE sync wait
# per instruction; rejects any wait on Drain).
# ---------------------------------------------------------------------------


def _drain_and_barrier_split(self, tick_clock, wait_clock):
    g = tick_clock.global_clock
    for p in range(N_PROCS):
        tick = g.peek_next(p) - 1
        if tick <= 0:
            continue
        vc = VectorClock()
        vc.require_at_least(p, tick)
        nop = self.nc.sync.nop(nofuse=True, hint="tail_wait_split")
        wait_clock.add_sem_waits(nop.ins, ScopedClock({None: vc}))

    self.nc.sync.drain()

    self.nc.all_engine_barrier()
    assert self.sems is not None
    popped = self.nc._tile_sem_poison_stack.pop()
    assert popped is self._sem_poison
    self.nc.clear_and_free_semaphores(list(self.sems.allocated().values()))
    self.nc.all_engine_barrier()


_orig_to_json_bytes = bass.Bass.to_json_bytes
_split_ctr = [0]


def _to_json_bytes_split(self):
    raw = _orig_to_json_bytes(self)
    m = json.loads(raw)
    changed = False
    for fn in m.get("functions", []):
        for blk in fn.get("blocks", []):
            insts = blk.get("instructions")
            if not insts:
                continue
            out = []
            for inst in insts:
                si = inst.get("sync_info")
                ow = (si or {}).get("on_wait") or []
                eng = inst.get("engine")
                if len(ow) > 1 and eng:
                    changed = True
                    for w in ow[:-1]:
                        _split_ctr[0] += 1
                        nop = {
                            "engine": eng,
                            "ins": [],
                            "outs": [],
                            "name": f"I-wsplit-{_split_ctr[0]}",
                            "opcode": "NoOp",
                            "sync_info": {"on_update": [], "on_wait": [w]},
                            "text_hint": "wait_split",
                        }
                        if inst.get("debug") is not None:
                            nop["debug"] = inst["debug"]
                        out.append(nop)
                    si["on_wait"] = [ow[-1]]
                out.append(inst)
            blk["instructions"] = out
    return json.dumps(m).encode() if changed else raw


_patched = False


def _install_patches():
    global _patched
    if _patched:
        return
    tile.TileContext._drain_and_barrier = _drain_and_barrier_split
    bass.Bass.to_json_bytes = _to_json_bytes_split
    # Calibrate the scheduler's cost model to measured HW rates: ACT and DVE
    # run slower than the stock model (per-op overheads), which otherwise
    # makes the static PE instruction stream stall on weight-production.
    from concourse.hw_specs import TRN2Spec

    TRN2Spec.CYCLE_T = {
        **TRN2Spec.CYCLE_T,
        mybir.EngineType.DVE: 1e9 / 0.96e9 * 1.4,
        mybir.EngineType.Activation: 1e9 / 1.2e9 * 1.9,
    }
    _patched = True


# ---------------------------------------------------------------------------
# Problem constants (hardcoded per contest rules)
# ---------------------------------------------------------------------------

T_FULL, O_FULL, I_FULL, G = 8192, 4096, 4096, 128
N_OSH, N_TSH = 4, 2  # o-quarters x t-halves = 8 cores
O_SH, T_SH = O_FULL // N_OSH, T_FULL // N_TSH  # 1024, 4096
N_OC = O_SH // 512  # 512-wide output chunks per core (2)
NK = I_FULL // 128  # 32 contraction chunks of 128
N_IC = 8  # i-chunks of 512 in phase A
N_OB = O_SH // 128  # 8 o-blocks per core
N_TT = T_SH // 128  # 32 t-tiles per core

f32 = mybir.dt.float32
f32r = mybir.dt.float32r
i32 = mybir.dt.int32
bf16 = mybir.dt.bfloat16

AF = mybir.ActivationFunctionType
ALU = mybir.AluOpType


def build_program(bench_loop=None, phases=("A", "B")) -> bass.Bass:
    _install_patches()
    nc = bass.Bass()
    if bench_loop is None:
        xT = nc.declare_dram_parameter("xT", [I_FULL, T_SH], f32r, isOutput=False)
        logits = nc.declare_dram_parameter("logits", [O_SH, I_FULL, 2], f32, isOutput=False)
        scales = nc.declare_dram_parameter("scales", [O_SH, I_FULL // G], f32, isOutput=False)
        mask = nc.declare_dram_parameter("mask", [O_SH, I_FULL], i32, isOutput=False)
        y = nc.declare_dram_parameter("y", [T_SH, O_SH], f32, isOutput=True)
    else:
        # timing-bench build: no host I/O of the big tensors, body looped
        dummy = nc.declare_dram_parameter("bench_in", [128, 1], f32, isOutput=False)
        dout = nc.declare_dram_parameter("bench_out", [128, 1], f32, isOutput=True)
        xT = nc.dram_tensor("xT_i", [I_FULL, T_SH], f32r)
        logits = nc.dram_tensor("logits_i", [O_SH, I_FULL, 2], f32)
        scales = nc.dram_tensor("scales_i", [O_SH, I_FULL // G], f32)
        mask = nc.dram_tensor("mask_i", [O_SH, I_FULL], i32)
        y = nc.dram_tensor("y_i", [T_SH, O_SH], f32)

    xT_t = xT.rearrange("(k p) t -> p k t", p=128)  # [128, NK, T_SH]
    scales_t = scales.rearrange("(ob p) g -> p ob g", p=128)  # [128, N_OB, 32]

    with tile.TileContext(nc) as tc:
        with (
            tc.tile_pool(name="persist", bufs=1) as persist,
            tc.tile_pool(name="wt", bufs=1) as wt_pool,
            tc.tile_pool(name="wa", bufs=2) as wa,
            tc.tile_pool(name="xin", bufs=2) as xin,
            tc.tile_pool(name="yout", bufs=2) as yout,
            tc.tile_pool(name="psa", bufs=3, space="PSUM") as psa,
            tc.tile_pool(name="psb", bufs=4, space="PSUM") as psb,
        ):
            ident = persist.tile([128, 128], f32)
            make_identity(nc, ident)

            # scales for all o-blocks, rounded through bf16 once
            s_raw = persist.tile([128, N_OB, 32], f32, tag="sraw")
            nc.sync.dma_start(out=s_raw, in_=scales_t)
            s_bf = persist.tile([128, N_OB, 32], bf16, tag="sbf")
            nc.vector.tensor_copy(out=s_bf, in_=s_raw)
            s_r = persist.tile([128, N_OB, 32], f32, tag="sr")
            nc.vector.tensor_copy(out=s_r, in_=s_bf)

            # resident transposed weights, one tile per 512-wide i-chunk
            # (layout [128 part, 4 k-quarters, O_SH])
            wT = [
                wt_pool.tile([128, 4, O_SH], f32r, tag=f"wT{ic}", name=f"wT{ic}")
                for ic in range(N_IC)
            ]

            if "A" not in phases:
                for ic in range(N_IC):
                    nc.vector.memset(wT[ic].bitcast(f32), 0.0)

            if bench_loop is not None:
                dcp = persist.tile([128, 1], f32, tag="dcp")
                nc.sync.dma_start(out=dcp, in_=dummy[:, :])
                nc.sync.dma_start(out=dout[:, :], in_=dcp)

            import contextlib

            loop_cm = (
                tc.For_i(0, bench_loop, 1)
                if bench_loop is not None
                else contextlib.nullcontext()
            )
            with loop_cm:
                _emit_body(nc, tc, wa, xin, yout, psa, psb, wT, ident, s_r,
                           xT_t, logits, mask, y, phases)

    if bench_loop is not None:
        # tie dummy IO so the program has external IO
        pass
    return nc


def _emit_body(nc, tc, wa, xin, yout, psa, psb, wT, ident, s_r, xT_t, logits, mask, y, phases=("A", "B")):
            # ---- phase A: weights (ic-outer so wT[ic] complete early) ----
            for ic in range(N_IC if "A" in phases else 0):
                for ob in range(N_OB):
                    L = wa.tile([128, 512, 2], f32, tag="L", bufs=4)
                    nc.sync.dma_start(
                        out=L, in_=logits[ob * 128 : (ob + 1) * 128, ic * 512 : (ic + 1) * 512, :]
                    )
                    M = wa.tile([128, 512], i32, tag="M", bufs=4)
                    nc.sync.dma_start(
                        out=M, in_=mask[ob * 128 : (ob + 1) * 128, ic * 512 : (ic + 1) * 512]
                    )
                    # E = exp(logits), in place
                    Lf = L.rearrange("p i s -> p (i s)")
                    nc.scalar.activation(out=Lf, in_=Lf, func=AF.Exp)
                    # D = e0 + 1 + e1
                    D = wa.tile([128, 512], f32, tag="D")
                    nc.vector.scalar_tensor_tensor(
                        out=D, in0=L[:, :, 0], scalar=1.0, in1=L[:, :, 1],
                        op0=ALU.add, op1=ALU.add,
                    )
                    # D <- 1/D via exp(-ln D)  (ScalarE; DVE reciprocal is slow)
                    if "norecip" not in phases:
                        nc.scalar.activation(out=D, in_=D, func=AF.Ln)
                        nc.scalar.activation(out=D, in_=D, func=AF.Exp, scale=-1.0)
                    # N = e1 - e0
                    N = wa.tile([128, 512], f32, tag="N")
                    nc.vector.tensor_tensor(
                        out=N, in0=L[:, :, 1], in1=L[:, :, 0], op=ALU.subtract
                    )
                    # N <- N * mask  (DVE auto-casts the int32 operand)
                    nc.vector.tensor_tensor(out=N, in0=N, in1=M, op=ALU.mult)
                    # N <- N * s_g  (stride-0 broadcast of the 4 group scales)
                    s_sl = s_r[:, ob, ic * 4 : (ic + 1) * 4]
                    s_bc = bass.AP(
                        tensor=s_sl.tensor,
                        offset=s_sl.offset,
                        ap=[s_sl.ap[0], s_sl.ap[1], [0, 128]],
                    )
                    N3 = N.rearrange("p (g c) -> p g c", g=4)
                    nc.vector.tensor_tensor(out=N3, in0=N3, in1=s_bc, op=ALU.mult)
                    # N <- N * (1/D)
                    nc.vector.tensor_tensor(out=N, in0=N, in1=D, op=ALU.mult)
                    # transpose 4x 128x128 blocks into one PSUM bank, then
                    # evacuate all four with a single ScalarE copy
                    pt = psa.tile([128, 512], f32, tag="pt")
                    for q in range(4):
                        nc.tensor.transpose(
                            out=pt[:, q * 128 : (q + 1) * 128],
                            in_=N[:, q * 128 : (q + 1) * 128],
                            identity=ident,
                        )
                    nc.vector.tensor_copy(
                        out=wT[ic][:, :, ob * 128 : (ob + 1) * 128],
                        in_=pt.rearrange("p (q c) -> p q c", q=4),
                    )

            # ---- phase B: y[t, o] = sum_k xT[k,t].T @ wT[k][:, o] ----
            # Accumulation is split into 4 quarter-groups of 8 k-chunks
            # (2 i-chunks each) so PSUM tiles close and recycle as soon as
            # early weight chunks exist -- that lets phase B's matmuls fill
            # the TensorE pipe while later weights are still being built.
            for tt in range(N_TT if "B" in phases else 0):
                xTt = xin.tile([128, NK, 128], f32r, tag="xTt")
                nc.sync.dma_start(
                    out=xTt, in_=xT_t[:, :, tt * 128 : (tt + 1) * 128]
                )
                pbs = [psb.tile([128, 512], f32, tag="pb", name=f"pb{oc}") for oc in range(N_OC)]
                for k in range(NK):
                    ic, q = divmod(k, 4)
                    for oc in range(N_OC):
                        nc.tensor.matmul(
                            out=pbs[oc],
                            lhsT=xTt[:, k, :],
                            rhs=wT[ic][:, q, oc * 512 : (oc + 1) * 512],
                            start=(k == 0),
                            stop=(k == NK - 1),
                        )
                y_sb = yout.tile([128, O_SH], f32, tag="ysb", bufs=3)
                for oc in range(N_OC):
                    nc.vector.tensor_copy(
                        out=y_sb[:, oc * 512 : (oc + 1) * 512], in_=pbs[oc]
                    )
                nc.sync.dma_start(
                    out=y[tt * 128 : (tt + 1) * 128, :], in_=y_sb
                )


_prog = None


def _get_program() -> bass.Bass:
    global _prog
    if _prog is None:
        _prog = build_program()
    return _prog


def kernel(x, logits, scales, mask):
    nc = _get_program()
    x = np.asarray(x, dtype=np.float32)
    logits = np.asarray(logits, dtype=np.float32)
    scales = np.asarray(scales, dtype=np.float32)
    mask_i = np.asarray(mask, dtype=np.int32)

    xT = np.ascontiguousarray(x.T)  # [I, T]
    in_maps = []
    for c in range(8):
        th, oq = divmod(c, N_OSH)
        in_maps.append(
            {
                "xT": np.ascontiguousarray(xT[:, th * T_SH : (th + 1) * T_SH]),
                "logits": np.ascontiguousarray(logits[oq * O_SH : (oq + 1) * O_SH]),
                "scales": np.ascontiguousarray(scales[oq * O_SH : (oq + 1) * O_SH]),
                "mask": np.ascontiguousarray(mask_i[oq * O_SH : (oq + 1) * O_SH]),
            }
        )
    res = run_bass_kernel_spmd(nc, in_maps, core_ids=list(range(8)))
    yf = np.empty((T_FULL, O_FULL), dtype=np.float32)
    for c in range(8):
        th, oq = divmod(c, N_OSH)
        yf[th * T_SH : (th + 1) * T_SH, oq * O_SH : (oq + 1) * O_SH] = res.results[c][
            "y"
        ]
    return yf



# revision 17
# speedup vs baseline: 1.4897x; 1.4897x over previous
"""Trainium2 Bass kernel for nn_MirrorDescentLinear.

Reference computation:
    w[o,i] = (e1 - e0) / (1 + e0 + e1)            (softmax(+1) - softmax(-1))
    w *= bf16(scales)[o, i//128]                   (per-group scale)
    w *= mask[o,i]                                 (0/1 int mask)
    y = x @ w.T                                    (f32, [8192,4096]@[4096,4096].T)

Sharding (8 cores): tensor-parallel 8-way on out_features (O_SH=512/core),
x replicated. Host pre-transposes x once and casts operands to bf16
(layout/dtype staging only; all module math stays on device). End-to-end
rel err ~5e-3 vs the f32 reference, within the 2e-2 gate.

Per-core device program, all-bf16 weight pipeline:
  phase A (weights, per 512-wide i-chunk x 128-row o-block):
    exp on ScalarE (bf16); D = 1+e0+e1 (DVE, f32); R = 1/D via the
    fast custom-DVE reciprocal; N = e1-e0 (DVE bf16 2x mode); mask and
    group-scale multiplies offloaded to GpSimd (Pool) so DVE/ACT/Pool
    each carry ~75-90us; N *= R on DVE; bf16 PE transposes (1 cyc/row)
    into a half-bank PSUM tile; ScalarE evacuates to resident bf16
    wT[ic] tiles.
  phase B (matmul): 64 t-tiles of y[128, 512] accumulate 32 k-chunks
    in PSUM (waves of 7 resident banks; 1 bank reserved for phase A
    transposes). Wave 0 is emitted interleaved with phase A's i-chunk
    loop so the PE consumes each wT[ic] as soon as it exists instead
    of idling behind weight production. bf16 matmuls at full rate
    (N=512 moving, fast weight load), f32 PSUM accumulation.
"""

import json
import sys

sys.path.insert(0, "/opt/trn_rl_repo")

import numpy as np

import concourse.bass as bass
import concourse.tile as tile
from concourse import mybir
from concourse.bass_utils import run_bass_kernel_spmd
from concourse.masks import make_identity
from concourse.tile_scheduler import N_PROCS
from concourse.vector_clock import ScopedClock, VectorClock

# ---------------------------------------------------------------------------
# Compatibility patches for the bundled walrus (accepts at most ONE sync wait
# per instruction; rejects any wait on Drain).
# ---------------------------------------------------------------------------


def _drain_and_barrier_split(self, tick_clock, wait_clock):
    g = tick_clock.global_clock
    for p in range(N_PROCS):
        tick = g.peek_next(p) - 1
        if tick <= 0:
            continue
        vc = VectorClock()
        vc.require_at_least(p, tick)
        nop = self.nc.sync.nop(nofuse=True, hint="tail_wait_split")
        wait_clock.add_sem_waits(nop.ins, ScopedClock({None: vc}))

    self.nc.sync.drain()

    self.nc.all_engine_barrier()
    assert self.sems is not None
    popped = self.nc._tile_sem_poison_stack.pop()
    assert popped is self._sem_poison
    self.nc.clear_and_free_semaphores(list(self.sems.allocated().values()))
    self.nc.all_engine_barrier()


_orig_to_json_bytes = bass.Bass.to_json_bytes
_split_ctr = [0]


def _to_json_bytes_split(self):
    raw = _orig_to_json_bytes(self)
    m = json.loads(raw)
    changed = False
    for fn in m.get("functions", []):
        for blk in fn.get("blocks", []):
            insts = blk.get("instructions")
            if not insts:
                continue
            out = []
            for inst in insts:
                si = inst.get("sync_info")
                ow = (si or {}).get("on_wait") or []
                eng = inst.get("engine")
                if len(ow) > 1 and eng:
                    changed = True
                    for w in ow[:-1]:
                        _split_ctr[0] += 1
                        nop = {
                            "engine": eng,
                            "ins": [],
                            "outs": [],
                            "name": f"I-wsplit-{_split_ctr[0]}",
                            "opcode": "NoOp",
                            "sync_info": {"on_update": [], "on_wait": [w]},
                            "text_hint": "wait_split",
                        }
                        if inst.get("debug") is not None:
                            nop["debug"] = inst["debug"]
                        out.append(nop)
                    si["on_wait"] = [ow[-1]]
                out.append(inst)
            blk["instructions"] = out
    return json.dumps(m).encode() if changed else raw


_patched = False


def _install_patches():
    global _patched
    if _patched:
        return
    tile.TileContext._drain_and_barrier = _drain_and_barrier_split
    bass.Bass.to_json_bytes = _to_json_bytes_split
    # Calibrate the scheduler's cost model to measured HW rates: ACT and DVE
    # run slower than the stock model (per-op overheads), which otherwise
    # makes the static PE instruction stream stall on weight-production.
    from concourse.hw_specs import TRN2Spec

    TRN2Spec.CYCLE_T = {
        **TRN2Spec.CYCLE_T,
        mybir.EngineType.DVE: 1e9 / 0.96e9 * 1.4,
        mybir.EngineType.Activation: 1e9 / 1.2e9 * 1.9,
    }
    _patched = True


# ---------------------------------------------------------------------------
# Problem constants (hardcoded per contest rules)
# ---------------------------------------------------------------------------

T_FULL, O_FULL, I_FULL, G = 8192, 4096, 4096, 128
N_CORES = 8
O_SH = O_FULL // N_CORES  # 512 out-features per core
N_OB = O_SH // 128  # 4 o-blocks per core
N_IC = 8  # i-chunks of 512
NK = I_FULL // 128  # 32 contraction chunks of 128
N_TT = T_FULL // 128  # 64 t-tiles per core
XT_COLS = 256  # t-columns per xT DMA tile (512B inner runs for bf16)
N_XT = T_FULL // XT_COLS  # 32
WAVE = 7  # PSUM-resident t-tiles per phase-B wave

f32 = mybir.dt.float32
bf16 = mybir.dt.bfloat16

AF = mybir.ActivationFunctionType
ALU = mybir.AluOpType


def build_program(phases=("A", "B")) -> bass.Bass:
    _install_patches()
    nc = bass.Bass()
    xT = nc.declare_dram_parameter("xT", [I_FULL, T_FULL], bf16, isOutput=False)
    logits = nc.declare_dram_parameter("logits", [O_SH, 2, I_FULL], bf16, isOutput=False)
    scales = nc.declare_dram_parameter("scales", [O_SH, I_FULL // G], bf16, isOutput=False)
    mask = nc.declare_dram_parameter("mask", [O_SH, I_FULL], bf16, isOutput=False)
    y = nc.declare_dram_parameter("y", [T_FULL, O_SH], f32, isOutput=True)

    xT_t = xT.rearrange("(k p) t -> p k t", p=128)  # [128, NK, T_FULL]
    scales_t = scales.rearrange("(ob p) g -> p ob g", p=128)  # [128, N_OB, 32]

    with tile.TileContext(nc) as tc:
        with (
            tc.tile_pool(name="persist", bufs=1) as persist,
            tc.tile_pool(name="wt", bufs=1) as wt_pool,
            tc.tile_pool(name="wa", bufs=2) as wa,
            tc.tile_pool(name="xin", bufs=6) as xin,
            tc.tile_pool(name="yout", bufs=3) as yout,
            tc.tile_pool(name="psa", bufs=1, space="PSUM") as psa,
            tc.tile_pool(name="psb", bufs=WAVE, space="PSUM") as psb,
        ):
            identb = persist.tile([128, 128], bf16)
            make_identity(nc, identb)

            s_sb = persist.tile([128, N_OB, 32], bf16, tag="ssb")
            nc.sync.dma_start(out=s_sb, in_=scales_t)

            # resident transposed bf16 weights, one tile per 512-wide i-chunk
            # (layout [128 part, 4 k-quarters, O_SH])
            wT = [
                wt_pool.tile([128, 4, O_SH], bf16, tag=f"wT{ic}", name=f"wT{ic}")
                for ic in range(N_IC)
            ]

            xtiles = {}

            def load_xtile(xt):
                tl = xin.tile([128, NK, XT_COLS], bf16, tag="xt", name="xt")
                nc.sync.dma_start(out=tl, in_=xT_t[:, :, xt * XT_COLS : (xt + 1) * XT_COLS])
                xtiles[xt] = tl

            # Phase A is software-pipelined: stage S of tile n is emitted in
            # the same "step" as stage S-1 of tile n+1 so every cross-engine
            # dependency points backward in each in-order engine queue.
            # Stages (tile n = (ic, ob) = divmod(n, 4)):
            #   dma(+2 ahead) -> exp(ACT) -> D/R/sub(DVE) -> mask/scale(Pool)
            #   -> N*=R(DVE) -> transpose(PE) -> evac(ACT)
            st = {}  # n -> dict of live tiles

            def a_dma(n):
                ic, ob = divmod(n, N_OB)
                L = wa.tile([128, 2, 512], bf16, tag="L", bufs=6, name="L")
                nc.sync.dma_start(
                    out=L,
                    in_=logits[ob * 128 : (ob + 1) * 128, :, ic * 512 : (ic + 1) * 512],
                )
                M = wa.tile([128, 512], bf16, tag="M", bufs=6, name="M")
                nc.sync.dma_start(
                    out=M, in_=mask[ob * 128 : (ob + 1) * 128, ic * 512 : (ic + 1) * 512]
                )
                st[n] = {"L": L, "M": M}

            def a_exp(n):
                if "noexp" in phases:
                    return
                L = st[n]["L"]
                Ef = L.rearrange("p c i -> p (c i)")
                nc.scalar.activation(out=Ef, in_=Ef, func=AF.Exp)

            def a_dve1(n):
                if "nodve1" in phases:
                    st[n]["R"] = None
                    st[n]["N"] = None
                    return
                L = st[n]["L"]
                D = wa.tile([128, 512], f32, tag="D", bufs=2, name="D")
                nc.vector.scalar_tensor_tensor(
                    out=D, in0=L[:, 0, :], scalar=1.0, in1=L[:, 1, :],
                    op0=ALU.add, op1=ALU.add,
                )
                R = wa.tile([128, 512], f32, tag="R", bufs=4, name="R")
                nc.vector.reciprocal(out=R, in_=D)
                N = wa.tile([128, 512], bf16, tag="N", bufs=6, name="N")
                nc.vector.tensor_tensor(
                    out=N, in0=L[:, 1, :], in1=L[:, 0, :], op=ALU.subtract
                )
                st[n]["R"] = R
                st[n]["N"] = N

            def a_pool(n):
                if "nodve1" in phases:
                    return
                ic, ob = divmod(n, N_OB)
                N = st[n]["N"]
                peng = nc.vector if "dveall" in phases else nc.gpsimd
                if "nomask" not in phases:
                    peng.tensor_tensor(out=N, in0=N, in1=st[n]["M"], op=ALU.mult)
                s_sl = s_sb[:, ob, ic * 4 : (ic + 1) * 4]
                s_bc = bass.AP(
                    tensor=s_sl.tensor,
                    offset=s_sl.offset,
                    ap=[s_sl.ap[0], s_sl.ap[1], [0, 128]],
                )
                N3 = N.rearrange("p (g c) -> p g c", g=4)
                if "noscale" not in phases:
                    peng.tensor_tensor(out=N3, in0=N3, in1=s_bc, op=ALU.mult)

            def a_nr(n):
                if "nonr" in phases or "nodve1" in phases:
                    return
                N = st[n]["N"]
                nc.vector.tensor_tensor(out=N, in0=N, in1=st[n]["R"], op=ALU.mult)

            def a_tr(n):
                if "nodve1" in phases:
                    return
                N = st[n]["N"]
                pt = psa.tile([128, 512], bf16, tag="pt", name="pt")
                for q in range(4):
                    nc.tensor.transpose(
                        out=pt[:, q * 128 : (q + 1) * 128],
                        in_=N[:, q * 128 : (q + 1) * 128],
                        identity=identb,
                    )
                st[n]["pt"] = pt

            def a_evac(n):
                ic, ob = divmod(n, N_OB)
                pt = st[n]["pt"]
                nc.scalar.activation(
                    out=wT[ic][:, :, ob * 128 : (ob + 1) * 128],
                    in_=pt.rearrange("p (q c) -> p q c", q=4),
                    func=AF.Copy,
                )
                del st[n]

            def emit_B_mms(tt, pb, k_lo, k_hi, k_start=0, k_stop=NK - 1):
                xt, half = divmod(tt, 2)
                tl = xtiles[xt]
                for k in range(k_lo, k_hi):
                    ic, q = divmod(k, 4)
                    nc.tensor.matmul(
                        out=pb,
                        lhsT=tl[:, k, half * 128 : (half + 1) * 128],
                        rhs=wT[ic][:, q, :],
                        start=(k == k_start),
                        stop=(k == k_stop),
                    )

            def emit_B_close(tt, pb):
                y_sb = yout.tile([128, O_SH], f32, tag="ysb", name="ysb")
                nc.vector.tensor_copy(out=y_sb, in_=pb)
                nc.sync.dma_start(out=y[tt * 128 : (tt + 1) * 128, :], in_=y_sb)

            # ---- phase B overlap scheme ----
            # PSUM (7 banks) bounds how much matmul work can be in flight
            # while phase A is still producing weights. To keep the PE busy
            # through A's whole span, the first two waves (g0 = tts 0-6,
            # g1 = tts 7-13) split their k-accumulation in half: the ic0-3
            # partial is evacuated to SBUF (par[tt]) so the banks can be
            # reused, and the ic4-7 half is added to it at close time.
            # g0-H1 follows A's ic releases (staggered entry + backfill),
            # g1-H1 then runs as solid PE filler, g0-H2 consumes ic4-7 as
            # they release, and everything after A is plain full-k waves.
            # g1 is 5 t-tiles so only xT tiles 0-5 (= xin bufs) stay live
            # across both H2 passes; a 6th would deadlock the DMA queue.
            entry = [0, 0, 1, 1, 2, 2, 3]
            G1N = 5
            KH = NK // 2  # 16: k-chunks in each half
            if "A" not in phases:
                for ic in range(N_IC):
                    nc.vector.memset(wT[ic].bitcast(f32), 0.0)
            if "B" not in phases:
                def emit_B_mms(tt, pb, k_lo, k_hi, k_start=0, k_stop=NK - 1):
                    pass
                def emit_B_close(tt, pb):
                    pass
            par = {}

            def emit_B_par_evac(tt, pb):
                if "B" not in phases:
                    return
                p = wa.tile([128, O_SH], f32, tag="par", bufs=WAVE + G1N, name="par")
                nc.vector.tensor_copy(out=p, in_=pb)
                par[tt] = p

            def emit_B_close_add(tt, pb):
                if "B" not in phases:
                    return
                y_sb = yout.tile([128, O_SH], f32, tag="ysb", name="ysb")
                nc.vector.tensor_tensor(out=y_sb, in0=pb, in1=par[tt], op=ALU.add)
                nc.sync.dma_start(out=y[tt * 128 : (tt + 1) * 128, :], in_=y_sb)

            pbs0 = [psb.tile([128, O_SH], f32, tag="pb", name=f"pb{tt}") for tt in range(WAVE)]
            N_AT = N_IC * N_OB  # 32 A-tiles
            load_xtile(0)
            if "A" in phases:
                a_dma(0)
                a_dma(1)
            pbs_h2 = None
            for t in range(N_AT + 6):
                if "A" in phases:
                    if t + 2 < N_AT:
                        a_dma(t + 2)
                    if 0 <= t - 5 < N_AT and "notr" not in phases:
                        a_evac(t - 5)
                    if t < N_AT:
                        a_exp(t)
                    if t - 4 >= 0 and t - 4 < N_AT and "notr" not in phases:
                        a_tr(t - 4)
                    if t - 1 >= 0 and t - 1 < N_AT:
                        a_dve1(t - 1)
                    if t - 2 >= 0 and t - 2 < N_AT:
                        a_pool(t - 2)
                    if t - 3 >= 0 and t - 3 < N_AT:
                        a_nr(t - 3)
                # spread xT loads: one per 3 steps (g0 needs xt0-3, g1 xt3-5)
                xt_next = 1 + (t // 3)
                if t % 3 == 0 and xt_next <= 5:
                    load_xtile(xt_next)
                # B matmuls once wT[ic] is fully evacuated
                if t - 5 >= 0 and (t - 5) % N_OB == N_OB - 1:
                    ic = (t - 5) // N_OB
                    if ic < N_IC // 2:
                        # g0-H1: staggered entry over ic0-3
                        for tt in range(WAVE):
                            if entry[tt] <= ic:
                                emit_B_mms(
                                    tt, pbs0[tt], ic * 4, (ic + 1) * 4,
                                    k_start=entry[tt] * 4,
                                    k_stop=(KH - 1) if entry[tt] == 0 else (entry[tt] * 4 - 1),
                                )
                        if ic == N_IC // 2 - 1:
                            # backfill staggered tts, evac H1 partials, then
                            # g1-H1 as solid PE filler for A's second half
                            for tt in range(WAVE):
                                if entry[tt] > 0:
                                    emit_B_mms(
                                        tt, pbs0[tt], 0, entry[tt] * 4,
                                        k_start=entry[tt] * 4,
                                        k_stop=entry[tt] * 4 - 1,
                                    )
                                emit_B_par_evac(tt, pbs0[tt])
                            for tt in range(WAVE, WAVE + G1N):
                                pb = psb.tile([128, O_SH], f32, tag="pb", name="pb")
                                emit_B_mms(tt, pb, 0, KH, k_start=0, k_stop=KH - 1)
                                emit_B_par_evac(tt, pb)
                            pbs_h2 = [
                                psb.tile([128, O_SH], f32, tag="pb", name=f"pbh{tt}")
                                for tt in range(WAVE)
                            ]
                    else:
                        # g0-H2: consume ic4-7 as they release
                        for tt in range(WAVE):
                            emit_B_mms(
                                tt, pbs_h2[tt], ic * 4, (ic + 1) * 4,
                                k_start=KH, k_stop=NK - 1,
                            )
            for tt in range(WAVE):
                emit_B_close_add(tt, pbs_h2[tt])
            # g1-H2
            for tt in range(WAVE, WAVE + G1N):
                pb = psb.tile([128, O_SH], f32, tag="pb", name="pb")
                emit_B_mms(tt, pb, KH, NK, k_start=KH, k_stop=NK - 1)
                emit_B_close_add(tt, pb)

            # ---- remaining waves: straight-line tt-major accumulation ----
            next_load = 6
            for tt in range(WAVE + G1N, N_TT):
                xt, half = divmod(tt, 2)
                if half == 0 or tt == WAVE + G1N:
                    while next_load <= min(xt + 2, N_XT - 1):
                        load_xtile(next_load)
                        next_load += 1
                pb = psb.tile([128, O_SH], f32, tag="pb", name="pb")
                emit_B_mms(tt, pb, 0, NK)
                emit_B_close(tt, pb)

    return nc


_prog = None


def _get_program() -> bass.Bass:
    global _prog
    if _prog is None:
        _prog = build_program()
    return _prog


def kernel(x, logits, scales, mask):
    import ml_dtypes

    bf = ml_dtypes.bfloat16
    nc = _get_program()
    x = np.asarray(x, dtype=np.float32)
    logits = np.asarray(logits, dtype=np.float32)
    scales = np.asarray(scales, dtype=np.float32)
    mask_i = np.asarray(mask, dtype=np.int32)

    # host staging: layout + dtype casts only (all math stays on device)
    xT = np.ascontiguousarray(x.T).astype(bf)  # [I, T] bf16
    logits_d = np.ascontiguousarray(np.transpose(logits, (0, 2, 1))).astype(bf)  # [O, 2, I]
    scales_b = scales.astype(bf)
    mask_b = mask_i.astype(bf)

    in_maps = []
    for c in range(N_CORES):
        sl = slice(c * O_SH, (c + 1) * O_SH)
        in_maps.append(
            {
                "xT": xT,
                "logits": np.ascontiguousarray(logits_d[sl]),
                "scales": np.ascontiguousarray(scales_b[sl]),
                "mask": np.ascontiguousarray(mask_b[sl]),
            }
        )
    res = run_bass_kernel_spmd(nc, in_maps, core_ids=list(range(N_CORES)))
    yf = np.empty((T_FULL, O_FULL), dtype=np.float32)
    for c in range(N_CORES):
        yf[:, c * O_SH : (c + 1) * O_SH] = res.results[c]["y"]
    return yf


# revision 28
# speedup vs baseline: 1.5190x; 1.0196x over previous
"""Trainium2 Bass kernel for nn_MirrorDescentLinear.

Reference computation:
    w[o,i] = (e1 - e0) / (1 + e0 + e1)            (softmax(+1) - softmax(-1))
    w *= bf16(scales)[o, i//128]                   (per-group scale)
    w *= mask[o,i]                                 (0/1 int mask)
    y = x @ w.T                                    (f32, [8192,4096]@[4096,4096].T)

Sharding (8 cores): tensor-parallel 8-way on out_features (O_SH=512/core),
x replicated. Host pre-transposes x once and casts operands to bf16
(layout/dtype staging only; all module math stays on device). End-to-end
rel err ~5e-3 vs the f32 reference, within the 2e-2 gate.

Per-core device program, all-bf16 weight pipeline:
  phase A (weights): software-pipelined over 16 tiles of [128 o, 1024 i]
    so each in-order engine queue only sees backward dependencies:
    exp on ScalarE (bf16, in place); D = 1+e0+e1 (DVE STT, f32);
    R = 1/D (DVE reciprocal -> bf16); N = e1-e0 (DVE bf16 2x mode);
    mask (int8) and group-scale multiplies on GpSimd/Pool; N *= R on
    DVE; 8 bf16 PE transposes per tile into a full-bank PSUM tile;
    two evacuations (DVE + ScalarE; GpSimd cannot read PSUM) into
    resident bf16 wT[ic] tiles.
  phase B (matmul): 64 t-tiles of y[128, 512] accumulate 32 k-chunks in
    PSUM (7 banks; the 8th holds phase-A transposes). To keep the PE fed
    while A is still producing weights, the first two waves split their
    k-range: g0 (7 tts) follows A's four weight releases with staggered
    entry, parks its k0-15 partial in SBUF, and accumulates k16-31 as
    releases land; g1 (3 tts) runs its k0-15 as solid PE filler after
    the H1 evacuation. All other waves are plain full-k accumulations.
    bf16 matmuls (512-wide moving operand), f32 PSUM accumulation.
"""

import json
import sys

sys.path.insert(0, "/opt/trn_rl_repo")

import numpy as np

import concourse.bass as bass
import concourse.tile as tile
from concourse import mybir
from concourse.bass_utils import run_bass_kernel_spmd
from concourse.masks import make_identity
from concourse.tile_scheduler import N_PROCS
from concourse.vector_clock import ScopedClock, VectorClock

# ---------------------------------------------------------------------------
# Compatibility patches for the bundled walrus (accepts at most ONE sync wait
# per instruction; rejects any wait on Drain).
# ---------------------------------------------------------------------------


def _drain_and_barrier_split(self, tick_clock, wait_clock):
    g = tick_clock.global_clock
    for p in range(N_PROCS):
        tick = g.peek_next(p) - 1
        if tick <= 0:
            continue
        vc = VectorClock()
        vc.require_at_least(p, tick)
        nop = self.nc.sync.nop(nofuse=True, hint="tail_wait_split")
        wait_clock.add_sem_waits(nop.ins, ScopedClock({None: vc}))

    self.nc.sync.drain()

    self.nc.all_engine_barrier()
    assert self.sems is not None
    popped = self.nc._tile_sem_poison_stack.pop()
    assert popped is self._sem_poison
    self.nc.clear_and_free_semaphores(list(self.sems.allocated().values()))
    self.nc.all_engine_barrier()


_orig_to_json_bytes = bass.Bass.to_json_bytes
_split_ctr = [0]


def _to_json_bytes_split(self):
    raw = _orig_to_json_bytes(self)
    m = json.loads(raw)
    changed = False
    for fn in m.get("functions", []):
        for blk in fn.get("blocks", []):
            insts = blk.get("instructions")
            if not insts:
                continue
            out = []
            for inst in insts:
                si = inst.get("sync_info")
                ow = (si or {}).get("on_wait") or []
                eng = inst.get("engine")
                if len(ow) > 1 and eng:
                    changed = True
                    for w in ow[:-1]:
                        _split_ctr[0] += 1
                        nop = {
                            "engine": eng,
                            "ins": [],
                            "outs": [],
                            "name": f"I-wsplit-{_split_ctr[0]}",
                            "opcode": "NoOp",
                            "sync_info": {"on_update": [], "on_wait": [w]},
                            "text_hint": "wait_split",
                        }
                        if inst.get("debug") is not None:
                            nop["debug"] = inst["debug"]
                        out.append(nop)
                    si["on_wait"] = [ow[-1]]
                out.append(inst)
            blk["instructions"] = out
    return json.dumps(m).encode() if changed else raw


_patched = False


def _install_patches():
    global _patched
    if _patched:
        return
    tile.TileContext._drain_and_barrier = _drain_and_barrier_split
    bass.Bass.to_json_bytes = _to_json_bytes_split
    # Calibrate the scheduler's cost model to measured HW rates: ACT and DVE
    # run slower than the stock model (per-op overheads), which otherwise
    # makes the static PE instruction stream stall on weight-production.
    from concourse.hw_specs import TRN2Spec

    TRN2Spec.CYCLE_T = {
        **TRN2Spec.CYCLE_T,
        mybir.EngineType.DVE: 1e9 / 0.96e9 * 1.4,
        mybir.EngineType.Activation: 1e9 / 1.2e9 * 1.9,
    }
    _patched = True


# ---------------------------------------------------------------------------
# Problem constants (hardcoded per contest rules)
# ---------------------------------------------------------------------------

T_FULL, O_FULL, I_FULL, G = 8192, 4096, 4096, 128
N_CORES = 8
O_SH = O_FULL // N_CORES  # 512 out-features per core
N_OB = O_SH // 128  # 4 o-blocks per core
N_IC = 8  # i-chunks of 512
NK = I_FULL // 128  # 32 contraction chunks of 128
N_TT = T_FULL // 128  # 64 t-tiles per core
XT_COLS = 256  # t-columns per xT DMA tile (512B inner runs for bf16)
N_XT = T_FULL // XT_COLS  # 32
WAVE = 7  # PSUM-resident t-tiles per phase-B wave

f32 = mybir.dt.float32
bf16 = mybir.dt.bfloat16

AF = mybir.ActivationFunctionType
ALU = mybir.AluOpType


def build_program(phases=("A", "B")) -> bass.Bass:
    _install_patches()
    nc = bass.Bass()
    xT = nc.declare_dram_parameter("xT", [I_FULL, T_FULL], bf16, isOutput=False)
    logits = nc.declare_dram_parameter("logits", [O_SH, 2, I_FULL], bf16, isOutput=False)
    scales = nc.declare_dram_parameter("scales", [O_SH, I_FULL // G], bf16, isOutput=False)
    mask = nc.declare_dram_parameter("mask", [O_SH, I_FULL], mybir.dt.int8, isOutput=False)
    y = nc.declare_dram_parameter("y", [T_FULL, O_SH], f32, isOutput=True)

    xT_t = xT.rearrange("(k p) t -> p k t", p=128)  # [128, NK, T_FULL]
    scales_t = scales.rearrange("(ob p) g -> p ob g", p=128)  # [128, N_OB, 32]

    with tile.TileContext(nc) as tc:
        with (
            tc.tile_pool(name="persist", bufs=1) as persist,
            tc.tile_pool(name="wt", bufs=1) as wt_pool,
            tc.tile_pool(name="wa", bufs=2) as wa,
            tc.tile_pool(name="xin", bufs=5) as xin,
            tc.tile_pool(name="yout", bufs=3) as yout,
            tc.tile_pool(name="psa", bufs=1, space="PSUM") as psa,
            tc.tile_pool(name="psb", bufs=WAVE, space="PSUM") as psb,
        ):
            identb = persist.tile([128, 128], bf16)
            make_identity(nc, identb)

            s_sb = persist.tile([128, N_OB, 32], bf16, tag="ssb")
            nc.sync.dma_start(out=s_sb, in_=scales_t)

            # resident transposed bf16 weights, one tile per 512-wide i-chunk
            # (layout [128 part, 4 k-quarters, O_SH])
            wT = [
                wt_pool.tile([128, 4, O_SH], bf16, tag=f"wT{ic}", name=f"wT{ic}")
                for ic in range(N_IC)
            ]

            xtiles = {}

            def load_xtile(xt):
                tl = xin.tile([128, NK, XT_COLS], bf16, tag="xt", name="xt")
                nc.sync.dma_start(out=tl, in_=xT_t[:, :, xt * XT_COLS : (xt + 1) * XT_COLS])
                xtiles[xt] = tl

            # Phase A is software-pipelined: stage S of tile n is emitted in
            # the same "step" as stage S-1 of tile n+1 so every cross-engine
            # dependency points backward in each in-order engine queue.
            # A-tiles are 1024 i-elems wide (n = (ic2, ob) = divmod(n, 4),
            # ic2 covering wT[2*ic2] and wT[2*ic2+1]) to amortize per-op
            # fixed costs. Stages:
            #   dma(+2 ahead) -> exp(ACT) -> D/R/sub(DVE) -> mask/scale(Pool)
            #   -> N*=R(DVE, all-bf16 2x) -> 8 transposes(PE) -> 2 evacs
            #   (one ACT, one DVE so the PSUM bank drains in parallel)
            st = {}  # n -> dict of live tiles

            def a_dma(n):
                ic2, ob = divmod(n, N_OB)
                L = wa.tile([128, 2, 1024], bf16, tag="L", bufs=4, name="L")
                nc.sync.dma_start(
                    out=L,
                    in_=logits[ob * 128 : (ob + 1) * 128, :, ic2 * 1024 : (ic2 + 1) * 1024],
                )
                M = wa.tile([128, 1024], mybir.dt.int8, tag="M", bufs=5, name="M")
                nc.sync.dma_start(
                    out=M, in_=mask[ob * 128 : (ob + 1) * 128, ic2 * 1024 : (ic2 + 1) * 1024]
                )
                st[n] = {"L": L, "M": M}

            def a_exp(n):
                L = st[n]["L"]
                Ef = L.rearrange("p c i -> p (c i)")
                nc.scalar.activation(out=Ef, in_=Ef, func=AF.Exp)

            def a_dve1(n):
                L = st[n]["L"]
                D = wa.tile([128, 1024], f32, tag="D", bufs=2, name="D")
                nc.vector.scalar_tensor_tensor(
                    out=D, in0=L[:, 0, :], scalar=1.0, in1=L[:, 1, :],
                    op0=ALU.add, op1=ALU.add,
                )
                R = wa.tile([128, 1024], bf16, tag="R", bufs=3, name="R")
                with nc.allow_low_precision(reason="weights are bf16 anyway"):
                    nc.vector.reciprocal(out=R, in_=D)
                N = wa.tile([128, 1024], bf16, tag="N", bufs=4, name="N")
                nc.vector.tensor_tensor(
                    out=N, in0=L[:, 1, :], in1=L[:, 0, :], op=ALU.subtract
                )
                st[n]["R"] = R
                st[n]["N"] = N

            def a_pool(n):
                ic2, ob = divmod(n, N_OB)
                N = st[n]["N"]
                nc.gpsimd.tensor_tensor(out=N, in0=N, in1=st[n]["M"], op=ALU.mult)
                s_sl = s_sb[:, ob, ic2 * 8 : (ic2 + 1) * 8]
                s_bc = bass.AP(
                    tensor=s_sl.tensor,
                    offset=s_sl.offset,
                    ap=[s_sl.ap[0], s_sl.ap[1], [0, 128]],
                )
                N3 = N.rearrange("p (g c) -> p g c", g=8)
                nc.gpsimd.tensor_tensor(out=N3, in0=N3, in1=s_bc, op=ALU.mult)

            def a_nr(n):
                N = st[n]["N"]
                nc.vector.tensor_tensor(out=N, in0=N, in1=st[n]["R"], op=ALU.mult)

            def a_tr(n):
                N = st[n]["N"]
                pt = psa.tile([128, 1024], bf16, tag="pt", name="pt")
                for q in range(8):
                    nc.tensor.transpose(
                        out=pt[:, q * 128 : (q + 1) * 128],
                        in_=N[:, q * 128 : (q + 1) * 128],
                        identity=identb,
                    )
                st[n]["pt"] = pt

            def a_evac(n):
                ic2, ob = divmod(n, N_OB)
                pt = st[n]["pt"]
                for h, eng in zip((0, 1), ("dve", "act")):
                    src = pt[:, h * 512 : (h + 1) * 512].rearrange(
                        "p (q c) -> p q c", q=4
                    )
                    dst = wT[2 * ic2 + h][:, :, ob * 128 : (ob + 1) * 128]
                    if eng == "act":
                        nc.scalar.activation(out=dst, in_=src, func=AF.Copy)
                    elif eng == "dve":
                        nc.vector.tensor_copy(out=dst, in_=src)
                    else:
                        nc.gpsimd.tensor_copy(out=dst, in_=src)
                del st[n]

            def emit_B_mms(tt, pb, k_lo, k_hi, k_start=0, k_stop=NK - 1):
                xt, half = divmod(tt, 2)
                tl = xtiles[xt]
                for k in range(k_lo, k_hi):
                    ic, q = divmod(k, 4)
                    nc.tensor.matmul(
                        out=pb,
                        lhsT=tl[:, k, half * 128 : (half + 1) * 128],
                        rhs=wT[ic][:, q, :],
                        start=(k == k_start),
                        stop=(k == k_stop),
                    )

            def emit_B_close(tt, pb):
                y_sb = yout.tile([128, O_SH], f32, tag="ysb", name="ysb")
                nc.scalar.activation(out=y_sb, in_=pb, func=AF.Copy)
                nc.sync.dma_start(out=y[tt * 128 : (tt + 1) * 128, :], in_=y_sb)

            # ---- phase B overlap scheme ----
            # PSUM (7 banks) bounds how much matmul work can be in flight
            # while phase A is still producing weights. To keep the PE busy
            # through A's whole span, the first two waves (g0 = tts 0-6,
            # g1 = tts 7-13) split their k-accumulation in half: the ic0-3
            # partial is evacuated to SBUF (par[tt]) so the banks can be
            # reused, and the ic4-7 half is added to it at close time.
            # g0-H1 follows A's ic releases (staggered entry + backfill),
            # g1-H1 then runs as solid PE filler, g0-H2 consumes ic4-7 as
            # they release, and everything after A is plain full-k waves.
            # g1 is 3 t-tiles so only xT tiles 0-4 (= xin bufs) stay live
            # across both H2 passes; a 6th would deadlock the DMA queue.
            entry = [0, 0, 0, 1, 1, 1, 1]  # release index (0..3) each g0 tt joins at
            G1N = 3
            KH = NK // 2  # 16: k-chunks in each half (releases 0-1 vs 2-3)
            if "A" not in phases:
                for ic in range(N_IC):
                    nc.vector.memset(wT[ic].bitcast(f32), 0.0)
            if "B" not in phases:
                def emit_B_mms(tt, pb, k_lo, k_hi, k_start=0, k_stop=NK - 1):
                    pass
                def emit_B_close(tt, pb):
                    pass
            par = {}

            def emit_B_par_evac(tt, pb):
                if "B" not in phases:
                    return
                p = wa.tile([128, O_SH], f32, tag="par", bufs=WAVE + G1N, name="par")
                nc.scalar.activation(out=p, in_=pb, func=AF.Copy)
                par[tt] = p

            def emit_B_close_add(tt, pb):
                if "B" not in phases:
                    return
                y_sb = yout.tile([128, O_SH], f32, tag="ysb", name="ysb")
                nc.vector.tensor_tensor(out=y_sb, in0=pb, in1=par[tt], op=ALU.add)
                nc.sync.dma_start(out=y[tt * 128 : (tt + 1) * 128, :], in_=y_sb)

            pbs0 = [psb.tile([128, O_SH], f32, tag="pb", name=f"pb{tt}") for tt in range(WAVE)]
            N_AT = 4 * N_OB  # 16 A-tiles (1024-wide)
            KR = NK // 4  # 8 k-chunks per weight release
            if "A" in phases:
                a_dma(0)
                a_dma(1)
            load_xtile(0)
            pbs_h2 = None
            for t in range(N_AT + 6):
                if "A" in phases:
                    if t + 2 < N_AT:
                        a_dma(t + 2)
                    if 0 <= t - 5 < N_AT and "notr" not in phases:
                        a_evac(t - 5)
                    if t < N_AT:
                        a_exp(t)
                    if t - 4 >= 0 and t - 4 < N_AT and "notr" not in phases:
                        a_tr(t - 4)
                    if t - 1 >= 0 and t - 1 < N_AT:
                        a_dve1(t - 1)
                    if t - 2 >= 0 and t - 2 < N_AT:
                        a_pool(t - 2)
                    if t - 3 >= 0 and t - 3 < N_AT:
                        a_nr(t - 3)
                # spread xT loads so A's logits/mask DMAs keep priority
                if t % 3 == 2:
                    xt_next = 1 + (t - 2) // 3
                    if xt_next <= 4:
                        load_xtile(xt_next)
                # B matmuls once a weight release (2 wT tiles) is evacuated
                if t - 5 >= 0 and (t - 5) % N_OB == N_OB - 1:
                    r = (t - 5) // N_OB  # release 0..3, k-chunks [8r, 8r+8)
                    if r < 2:
                        # g0-H1: staggered entry over releases 0-1
                        for tt in range(WAVE):
                            if entry[tt] <= r:
                                emit_B_mms(
                                    tt, pbs0[tt], r * KR, (r + 1) * KR,
                                    k_start=entry[tt] * KR,
                                    k_stop=(KH - 1) if entry[tt] == 0 else (entry[tt] * KR - 1),
                                )
                        if r == 1:
                            # backfill staggered tts, evac H1 partials, then
                            # g1-H1 as solid PE filler for A's second half
                            for tt in range(WAVE):
                                if entry[tt] > 0:
                                    emit_B_mms(
                                        tt, pbs0[tt], 0, entry[tt] * KR,
                                        k_start=entry[tt] * KR,
                                        k_stop=entry[tt] * KR - 1,
                                    )
                                emit_B_par_evac(tt, pbs0[tt])
                            for tt in range(WAVE, WAVE + G1N):
                                pb = psb.tile([128, O_SH], f32, tag="pb", name="pb")
                                emit_B_mms(tt, pb, 0, KH, k_start=0, k_stop=KH - 1)
                                emit_B_par_evac(tt, pb)
                            pbs_h2 = [
                                psb.tile([128, O_SH], f32, tag="pb", name=f"pbh{tt}")
                                for tt in range(WAVE)
                            ]
                    else:
                        # g0-H2: consume releases 2-3 as they land
                        for tt in range(WAVE):
                            emit_B_mms(
                                tt, pbs_h2[tt], r * KR, (r + 1) * KR,
                                k_start=KH, k_stop=NK - 1,
                            )
            for tt in range(WAVE):
                emit_B_close_add(tt, pbs_h2[tt])
            # g1-H2
            for tt in range(WAVE, WAVE + G1N):
                pb = psb.tile([128, O_SH], f32, tag="pb", name="pb")
                emit_B_mms(tt, pb, KH, NK, k_start=KH, k_stop=NK - 1)
                emit_B_close_add(tt, pb)

            if "wdbg" in phases:
                wdbg = nc.declare_dram_parameter("wdbg", [N_IC, 128, 4, O_SH], bf16, isOutput=True)
                for ic in range(N_IC):
                    nc.sync.dma_start(out=wdbg[ic], in_=wT[ic])

            # ---- remaining waves: straight-line tt-major accumulation ----
            next_load = 5
            for tt in range(WAVE + G1N, N_TT):
                xt, half = divmod(tt, 2)
                if half == 0 or tt == WAVE + G1N:
                    while next_load <= min(xt + 2, N_XT - 1):
                        load_xtile(next_load)
                        next_load += 1
                pb = psb.tile([128, O_SH], f32, tag="pb", name="pb")
                emit_B_mms(tt, pb, 0, NK)
                emit_B_close(tt, pb)

    return nc


_prog = None


def _get_program() -> bass.Bass:
    global _prog
    if _prog is None:
        _prog = build_program()
    return _prog


def kernel(x, logits, scales, mask):
    import ml_dtypes

    bf = ml_dtypes.bfloat16
    nc = _get_program()
    x = np.asarray(x, dtype=np.float32)
    logits = np.asarray(logits, dtype=np.float32)
    scales = np.asarray(scales, dtype=np.float32)
    mask_i = np.asarray(mask, dtype=np.int32)

    # host staging: layout + dtype casts only (all math stays on device)
    xT = np.ascontiguousarray(x.T).astype(bf)  # [I, T] bf16
    logits_d = np.ascontiguousarray(np.transpose(logits, (0, 2, 1))).astype(bf)  # [O, 2, I]
    scales_b = scales.astype(bf)
    mask_b = mask_i.astype(np.int8)

    in_maps = []
    for c in range(N_CORES):
        sl = slice(c * O_SH, (c + 1) * O_SH)
        in_maps.append(
            {
                "xT": xT,
                "logits": np.ascontiguousarray(logits_d[sl]),
                "scales": np.ascontiguousarray(scales_b[sl]),
                "mask": np.ascontiguousarray(mask_b[sl]),
            }
        )
    res = run_bass_kernel_spmd(nc, in_maps, core_ids=list(range(N_CORES)))
    yf = np.empty((T_FULL, O_FULL), dtype=np.float32)
    for c in range(N_CORES):
        yf[:, c * O_SH : (c + 1) * O_SH] = res.results[c]["y"]
    return yf


# revision 40
# speedup vs baseline: 1.5355x; 1.0109x over previous
"""Trainium2 Bass kernel for nn_MirrorDescentLinear.

Reference computation:
    w[o,i] = (e1 - e0) / (1 + e0 + e1)            (softmax(+1) - softmax(-1))
    w *= bf16(scales)[o, i//128]                   (per-group scale)
    w *= mask[o,i]                                 (0/1 int mask)
    y = x @ w.T                                    (f32, [8192,4096]@[4096,4096].T)

Sharding (8 cores): tensor-parallel 8-way on out_features (O_SH=512/core),
x replicated. Host pre-transposes x once and casts operands to bf16
(layout/dtype staging only; all module math stays on device). End-to-end
rel err ~5e-3 vs the f32 reference, within the 2e-2 gate.

Per-core device program, all-bf16 weight pipeline:
  phase A (weights): software-pipelined over 16 tiles of [128 o, 1024 i]
    so each in-order engine queue only sees backward dependencies:
    exp on ScalarE (bf16, in place); D = 1+e0+e1 (DVE STT, f32);
    R = 1/D (DVE reciprocal -> bf16); N = e1-e0 (DVE bf16 2x mode);
    mask (int8) and group-scale multiplies on GpSimd/Pool; N *= R on
    DVE; 8 bf16 PE transposes per tile into a full-bank PSUM tile;
    two evacuations (DVE + ScalarE; GpSimd cannot read PSUM) into
    resident bf16 wT[ic] tiles.
  phase B (matmul): 64 t-tiles of y[128, 512] accumulate 32 k-chunks in
    PSUM (7 banks; the 8th holds phase-A transposes). To keep the PE fed
    while A is still producing weights, the first two waves split their
    k-range: g0 (7 tts) follows A's four weight releases with staggered
    entry, parks its k0-15 partial in SBUF, and accumulates k16-31 as
    releases land; g1 (3 tts) runs its k0-15 as solid PE filler after
    the H1 evacuation. All other waves are plain full-k accumulations.
    bf16 matmuls (512-wide moving operand), f32 PSUM accumulation.
"""

import json
import sys

sys.path.insert(0, "/opt/trn_rl_repo")

import numpy as np

import concourse.bass as bass
import concourse.tile as tile
from concourse import mybir
from concourse.bass_utils import run_bass_kernel_spmd
from concourse.masks import make_identity
from concourse.tile_scheduler import N_PROCS
from concourse.vector_clock import ScopedClock, VectorClock

# ---------------------------------------------------------------------------
# Compatibility patches for the bundled walrus (accepts at most ONE sync wait
# per instruction; rejects any wait on Drain).
# ---------------------------------------------------------------------------


def _drain_and_barrier_split(self, tick_clock, wait_clock):
    g = tick_clock.global_clock
    for p in range(N_PROCS):
        tick = g.peek_next(p) - 1
        if tick <= 0:
            continue
        vc = VectorClock()
        vc.require_at_least(p, tick)
        nop = self.nc.sync.nop(nofuse=True, hint="tail_wait_split")
        wait_clock.add_sem_waits(nop.ins, ScopedClock({None: vc}))

    self.nc.sync.drain()

    self.nc.all_engine_barrier()
    assert self.sems is not None
    popped = self.nc._tile_sem_poison_stack.pop()
    assert popped is self._sem_poison
    self.nc.clear_and_free_semaphores(list(self.sems.allocated().values()))
    self.nc.all_engine_barrier()


_orig_to_json_bytes = bass.Bass.to_json_bytes
_split_ctr = [0]


def _to_json_bytes_split(self):
    raw = _orig_to_json_bytes(self)
    m = json.loads(raw)
    changed = False
    for fn in m.get("functions", []):
        for blk in fn.get("blocks", []):
            insts = blk.get("instructions")
            if not insts:
                continue
            out = []
            for inst in insts:
                si = inst.get("sync_info")
                ow = (si or {}).get("on_wait") or []
                eng = inst.get("engine")
                if len(ow) > 1 and eng:
                    changed = True
                    for w in ow[:-1]:
                        _split_ctr[0] += 1
                        nop = {
                            "engine": eng,
                            "ins": [],
                            "outs": [],
                            "name": f"I-wsplit-{_split_ctr[0]}",
                            "opcode": "NoOp",
                            "sync_info": {"on_update": [], "on_wait": [w]},
                            "text_hint": "wait_split",
                        }
                        if inst.get("debug") is not None:
                            nop["debug"] = inst["debug"]
                        out.append(nop)
                    si["on_wait"] = [ow[-1]]
                out.append(inst)
            blk["instructions"] = out
    return json.dumps(m).encode() if changed else raw


_patched = False


def _install_patches():
    global _patched
    if _patched:
        return
    tile.TileContext._drain_and_barrier = _drain_and_barrier_split
    bass.Bass.to_json_bytes = _to_json_bytes_split
    # Calibrate the scheduler's cost model to measured HW rates: ACT and DVE
    # run slower than the stock model (per-op overheads), which otherwise
    # makes the static PE instruction stream stall on weight-production.
    from concourse.hw_specs import TRN2Spec

    TRN2Spec.CYCLE_T = {
        **TRN2Spec.CYCLE_T,
        mybir.EngineType.DVE: 1e9 / 0.96e9 * 1.4,
        mybir.EngineType.Activation: 1e9 / 1.2e9 * 1.9,
    }
    _patched = True


# ---------------------------------------------------------------------------
# Problem constants (hardcoded per contest rules)
# ---------------------------------------------------------------------------

T_FULL, O_FULL, I_FULL, G = 8192, 4096, 4096, 128
N_CORES = 8
O_SH = O_FULL // N_CORES  # 512 out-features per core
N_OB = O_SH // 128  # 4 o-blocks per core
N_IC = 8  # i-chunks of 512
NK = I_FULL // 128  # 32 contraction chunks of 128
N_TT = T_FULL // 128  # 64 t-tiles per core
XT_COLS = 256  # t-columns per xT DMA tile (512B inner runs for bf16)
N_XT = T_FULL // XT_COLS  # 32
WAVE = 7  # PSUM-resident t-tiles per phase-B wave

f32 = mybir.dt.float32
bf16 = mybir.dt.bfloat16

AF = mybir.ActivationFunctionType
ALU = mybir.AluOpType


def build_program(phases=("A", "B")) -> bass.Bass:
    _install_patches()
    nc = bass.Bass()
    xT = nc.declare_dram_parameter("xT", [I_FULL, T_FULL], bf16, isOutput=False)
    logits = nc.declare_dram_parameter("logits", [O_SH, 2, I_FULL], bf16, isOutput=False)
    scales = nc.declare_dram_parameter("scales", [O_SH, I_FULL // G], bf16, isOutput=False)
    mask = nc.declare_dram_parameter("mask", [O_SH, I_FULL], mybir.dt.int8, isOutput=False)
    y = nc.declare_dram_parameter("y", [T_FULL, O_SH], f32, isOutput=True)

    xT_t = xT.rearrange("(k p) t -> p k t", p=128)  # [128, NK, T_FULL]
    scales_t = scales.rearrange("(ob p) g -> p ob g", p=128)  # [128, N_OB, 32]

    with tile.TileContext(nc) as tc:
        with (
            tc.tile_pool(name="persist", bufs=1) as persist,
            tc.tile_pool(name="wt", bufs=1) as wt_pool,
            tc.tile_pool(name="wa", bufs=2) as wa,
            tc.tile_pool(name="xin", bufs=5) as xin,
            tc.tile_pool(name="yout", bufs=3) as yout,
            tc.tile_pool(name="psa", bufs=1, space="PSUM") as psa,
            tc.tile_pool(name="psb", bufs=WAVE, space="PSUM") as psb,
        ):
            identb = persist.tile([128, 128], bf16)
            make_identity(nc, identb)

            s_sb = persist.tile([128, N_OB, 32], bf16, tag="ssb")
            nc.sync.dma_start(out=s_sb, in_=scales_t)

            # resident transposed bf16 weights, one tile per 512-wide i-chunk
            # (layout [128 part, 4 k-quarters, O_SH])
            wT = [
                wt_pool.tile([128, 4, O_SH], bf16, tag=f"wT{ic}", name=f"wT{ic}")
                for ic in range(N_IC)
            ]

            xtiles = {}

            def load_xtile(xt):
                tl = xin.tile([128, NK, XT_COLS], bf16, tag="xt", name="xt")
                nc.sync.dma_start(out=tl, in_=xT_t[:, :, xt * XT_COLS : (xt + 1) * XT_COLS])
                xtiles[xt] = tl

            # Phase A is software-pipelined: stage S of tile n is emitted in
            # the same "step" as stage S-1 of tile n+1 so every cross-engine
            # dependency points backward in each in-order engine queue.
            # A-tiles are 1024 i-elems wide (n = (ic2, ob) = divmod(n, 4),
            # ic2 covering wT[2*ic2] and wT[2*ic2+1]) to amortize per-op
            # fixed costs. Stages:
            #   dma(+2 ahead) -> exp(ACT) -> D/R/sub(DVE) -> mask/scale(Pool)
            #   -> N*=R(DVE, all-bf16 2x) -> 8 transposes(PE) -> 2 evacs
            #   (one ACT, one DVE so the PSUM bank drains in parallel)
            st = {}  # n -> dict of live tiles

            def a_dma(n):
                ic2, ob = divmod(n, N_OB)
                L = wa.tile([128, 2, 1024], bf16, tag="L", bufs=4, name="L")
                nc.sync.dma_start(
                    out=L,
                    in_=logits[ob * 128 : (ob + 1) * 128, :, ic2 * 1024 : (ic2 + 1) * 1024],
                )
                M = wa.tile([128, 1024], mybir.dt.int8, tag="M", bufs=5, name="M")
                nc.sync.dma_start(
                    out=M, in_=mask[ob * 128 : (ob + 1) * 128, ic2 * 1024 : (ic2 + 1) * 1024]
                )
                st[n] = {"L": L, "M": M}

            def a_exp(n):
                L = st[n]["L"]
                Ef = L.rearrange("p c i -> p (c i)")
                nc.scalar.activation(out=Ef, in_=Ef, func=AF.Exp)

            def a_dve1(n):
                L = st[n]["L"]
                D = wa.tile([128, 1024], f32, tag="D", bufs=2, name="D")
                nc.vector.scalar_tensor_tensor(
                    out=D, in0=L[:, 0, :], scalar=1.0, in1=L[:, 1, :],
                    op0=ALU.add, op1=ALU.add,
                )
                R = wa.tile([128, 1024], bf16, tag="R", bufs=3, name="R")
                with nc.allow_low_precision(reason="weights are bf16 anyway"):
                    nc.vector.reciprocal(out=R, in_=D)
                N = wa.tile([128, 1024], bf16, tag="N", bufs=4, name="N")
                nc.vector.tensor_tensor(
                    out=N, in0=L[:, 1, :], in1=L[:, 0, :], op=ALU.subtract
                )
                st[n]["R"] = R
                st[n]["N"] = N

            def a_pool(n):
                ic2, ob = divmod(n, N_OB)
                N = st[n]["N"]
                nc.gpsimd.tensor_tensor(out=N, in0=N, in1=st[n]["M"], op=ALU.mult)
                s_sl = s_sb[:, ob, ic2 * 8 : (ic2 + 1) * 8]
                s_bc = bass.AP(
                    tensor=s_sl.tensor,
                    offset=s_sl.offset,
                    ap=[s_sl.ap[0], s_sl.ap[1], [0, 128]],
                )
                N3 = N.rearrange("p (g c) -> p g c", g=8)
                nc.gpsimd.tensor_tensor(out=N3, in0=N3, in1=s_bc, op=ALU.mult)

            def a_nr(n):
                N = st[n]["N"]
                nc.vector.tensor_tensor(out=N, in0=N, in1=st[n]["R"], op=ALU.mult)

            def a_tr(n):
                N = st[n]["N"]
                pt = psa.tile([128, 1024], bf16, tag="pt", name="pt")
                for q in range(8):
                    nc.tensor.transpose(
                        out=pt[:, q * 128 : (q + 1) * 128],
                        in_=N[:, q * 128 : (q + 1) * 128],
                        identity=identb,
                    )
                st[n]["pt"] = pt

            def a_evac(n):
                ic2, ob = divmod(n, N_OB)
                pt = st[n]["pt"]
                engs = ("dve", "dve") if n < 4 else ("dve", "act")
                for h, eng in zip((0, 1), engs):
                    src = pt[:, h * 512 : (h + 1) * 512].rearrange(
                        "p (q c) -> p q c", q=4
                    )
                    dst = wT[2 * ic2 + h][:, :, ob * 128 : (ob + 1) * 128]
                    if eng == "act":
                        nc.scalar.activation(out=dst, in_=src, func=AF.Copy)
                    elif eng == "dve":
                        nc.vector.tensor_copy(out=dst, in_=src)
                    else:
                        nc.gpsimd.tensor_copy(out=dst, in_=src)
                del st[n]

            def emit_B_mms(tt, pb, k_lo, k_hi, k_start=0, k_stop=NK - 1):
                xt, half = divmod(tt, 2)
                tl = xtiles[xt]
                for k in range(k_lo, k_hi):
                    ic, q = divmod(k, 4)
                    nc.tensor.matmul(
                        out=pb,
                        lhsT=tl[:, k, half * 128 : (half + 1) * 128],
                        rhs=wT[ic][:, q, :],
                        start=(k == k_start),
                        stop=(k == k_stop),
                    )

            def emit_B_close(tt, pb):
                y_sb = yout.tile([128, O_SH], f32, tag="ysb", name="ysb")
                nc.scalar.activation(out=y_sb, in_=pb, func=AF.Copy)
                nc.sync.dma_start(out=y[tt * 128 : (tt + 1) * 128, :], in_=y_sb)

            # ---- phase B overlap scheme ----
            # PSUM (7 banks) bounds how much matmul work can be in flight
            # while phase A is still producing weights. To keep the PE busy
            # through A's whole span, the first two waves (g0 = tts 0-6,
            # g1 = tts 7-13) split their k-accumulation in half: the ic0-3
            # partial is evacuated to SBUF (par[tt]) so the banks can be
            # reused, and the ic4-7 half is added to it at close time.
            # g0-H1 follows A's ic releases (staggered entry + backfill),
            # g1-H1 then runs as solid PE filler, g0-H2 consumes ic4-7 as
            # they release, and everything after A is plain full-k waves.
            # g1 is 3 t-tiles so only xT tiles 0-4 (= xin bufs) stay live
            # across both H2 passes; a 6th would deadlock the DMA queue.
            entry = [0, 0, 0, 0, 0, 1, 1]  # release index (0..3) each g0 tt joins at
            G1N = 3
            KH = NK // 2  # 16: k-chunks in each half (releases 0-1 vs 2-3)
            if "A" not in phases:
                for ic in range(N_IC):
                    nc.vector.memset(wT[ic].bitcast(f32), 0.0)
            if "B" not in phases:
                def emit_B_mms(tt, pb, k_lo, k_hi, k_start=0, k_stop=NK - 1):
                    pass
                def emit_B_close(tt, pb):
                    pass
            par = {}

            def emit_B_par_evac(tt, pb):
                if "B" not in phases:
                    return
                p = wa.tile([128, O_SH], f32, tag="par", bufs=WAVE + G1N, name="par")
                if "parDVE" in phases:
                    nc.vector.tensor_copy(out=p, in_=pb)
                else:
                    nc.scalar.activation(out=p, in_=pb, func=AF.Copy)
                par[tt] = p

            def emit_B_close_add(tt, pb):
                if "B" not in phases:
                    return
                y_sb = yout.tile([128, O_SH], f32, tag="ysb", name="ysb")
                nc.vector.tensor_tensor(out=y_sb, in0=pb, in1=par[tt], op=ALU.add)
                nc.sync.dma_start(out=y[tt * 128 : (tt + 1) * 128, :], in_=y_sb)

            pbs0 = [psb.tile([128, O_SH], f32, tag="pb", name=f"pb{tt}") for tt in range(WAVE)]
            N_AT = 4 * N_OB  # 16 A-tiles (1024-wide)
            KR = NK // 4  # 8 k-chunks per weight release
            if "A" in phases:
                a_dma(0)
                a_dma(1)
            load_xtile(0)
            pbs_h2 = None
            for t in range(N_AT + 6):
                if "A" in phases:
                    if t + 2 < N_AT:
                        a_dma(t + 2)
                    if 0 <= t - 5 < N_AT and "notr" not in phases:
                        a_evac(t - 5)
                    if t < N_AT:
                        a_exp(t)
                    if t - 4 >= 0 and t - 4 < N_AT and "notr" not in phases:
                        a_tr(t - 4)
                    if t - 1 >= 0 and t - 1 < N_AT:
                        a_dve1(t - 1)
                    if t - 2 >= 0 and t - 2 < N_AT:
                        a_pool(t - 2)
                    if t - 3 >= 0 and t - 3 < N_AT:
                        a_nr(t - 3)
                # spread xT loads so A's logits/mask DMAs keep priority
                if t % 3 == 2:
                    xt_next = 1 + (t - 2) // 3
                    if xt_next <= 4:
                        load_xtile(xt_next)
                # B matmuls once a weight release (2 wT tiles) is evacuated
                if t - 5 >= 0 and (t - 5) % N_OB == N_OB - 1:
                    r = (t - 5) // N_OB  # release 0..3, k-chunks [8r, 8r+8)
                    if r < 2:
                        # g0-H1: staggered entry over releases 0-1 (k-major so
                        # work gated on wT[2r] starts before wT[2r+1] lands)
                        for k in range(r * KR, (r + 1) * KR):
                            for tt in range(WAVE):
                                if entry[tt] <= r:
                                    emit_B_mms(
                                        tt, pbs0[tt], k, k + 1,
                                        k_start=entry[tt] * KR,
                                        k_stop=(KH - 1) if entry[tt] == 0 else (entry[tt] * KR - 1),
                                    )
                        if r == 1:
                            # backfill staggered tts, evac H1 partials, then
                            # g1-H1 as solid PE filler for A's second half
                            for tt in range(WAVE):
                                if entry[tt] > 0:
                                    emit_B_mms(
                                        tt, pbs0[tt], 0, entry[tt] * KR,
                                        k_start=entry[tt] * KR,
                                        k_stop=entry[tt] * KR - 1,
                                    )
                                emit_B_par_evac(tt, pbs0[tt])
                            for tt in range(WAVE, WAVE + G1N):
                                pb = psb.tile([128, O_SH], f32, tag="pb", name="pb")
                                emit_B_mms(tt, pb, 0, KH, k_start=0, k_stop=KH - 1)
                                emit_B_par_evac(tt, pb)
                            pbs_h2 = [
                                psb.tile([128, O_SH], f32, tag="pb", name=f"pbh{tt}")
                                for tt in range(WAVE)
                            ]
                    else:
                        # g0-H2: consume releases 2-3 as they land (k-major)
                        for k in range(r * KR, (r + 1) * KR):
                            for tt in range(WAVE):
                                emit_B_mms(
                                    tt, pbs_h2[tt], k, k + 1,
                                    k_start=KH, k_stop=NK - 1,
                                )
            for tt in range(WAVE):
                emit_B_close_add(tt, pbs_h2[tt])
            # g1-H2
            for tt in range(WAVE, WAVE + G1N):
                pb = psb.tile([128, O_SH], f32, tag="pb", name="pb")
                emit_B_mms(tt, pb, KH, NK, k_start=KH, k_stop=NK - 1)
                emit_B_close_add(tt, pb)

            if "wdbg" in phases:
                wdbg = nc.declare_dram_parameter("wdbg", [N_IC, 128, 4, O_SH], bf16, isOutput=True)
                for ic in range(N_IC):
                    nc.sync.dma_start(out=wdbg[ic], in_=wT[ic])

            # ---- remaining waves: straight-line tt-major accumulation ----
            next_load = 5
            for tt in range(WAVE + G1N, N_TT):
                xt, half = divmod(tt, 2)
                if half == 0 or tt == WAVE + G1N:
                    while next_load <= min(xt + 2, N_XT - 1):
                        load_xtile(next_load)
                        next_load += 1
                pb = psb.tile([128, O_SH], f32, tag="pb", name="pb")
                emit_B_mms(tt, pb, 0, NK)
                emit_B_close(tt, pb)

    return nc


_prog = None


def _get_program() -> bass.Bass:
    global _prog
    if _prog is None:
        _prog = build_program()
    return _prog


def kernel(x, logits, scales, mask):
    import ml_dtypes

    bf = ml_dtypes.bfloat16
    nc = _get_program()
    x = np.asarray(x, dtype=np.float32)
    logits = np.asarray(logits, dtype=np.float32)
    scales = np.asarray(scales, dtype=np.float32)
    mask_i = np.asarray(mask, dtype=np.int32)

    # host staging: layout + dtype casts only (all math stays on device)
    xT = np.ascontiguousarray(x.T).astype(bf)  # [I, T] bf16
    logits_d = np.ascontiguousarray(np.transpose(logits, (0, 2, 1))).astype(bf)  # [O, 2, I]
    scales_b = scales.astype(bf)
    mask_b = mask_i.astype(np.int8)

    in_maps = []
    for c in range(N_CORES):
        sl = slice(c * O_SH, (c + 1) * O_SH)
        in_maps.append(
            {
                "xT": xT,
                "logits": np.ascontiguousarray(logits_d[sl]),
                "scales": np.ascontiguousarray(scales_b[sl]),
                "mask": np.ascontiguousarray(mask_b[sl]),
            }
        )
    res = run_bass_kernel_spmd(nc, in_maps, core_ids=list(range(N_CORES)))
    yf = np.empty((T_FULL, O_FULL), dtype=np.float32)
    for c in range(N_CORES):
        yf[:, c * O_SH : (c + 1) * O_SH] = res.results[c]["y"]
    return yf
